# revision 2
# baseline (speedup 1.0000x reference)
"""Trainium2 Bass kernel for nn_MessagePassingNN (gnn_message_passing). v3.

B, N, F, H, A, T = 4, 256, 64, 256, 16, 3

Sharding: 8 cores = (batch b = c//2, receiver-half c%2). Node indexing is
core-relative ([my 128 | partner 128]); host permutes inputs per core.

v3 changes vs the 405-472us v2 baseline (trace: SC+DVE both ~90% busy on the
e-loop two-op path; ~30us V/S idle bubble per iteration boundary):
- e-loop uses a RUNTIME-REGISTERED custom DVE op RELU_MASK_ACC_ANT:
    accum_out = s1 + sum_j relu(hjbT[h,j] + hiT[h,i]) * adj01[i,j]
  ONE DVE instruction per (receiver, h-tile) replaces the TT-mask +
  STT/ScalarE-activation pair. adj mask rides in1 as 0/1 bf16 broadcast.
- Hybrid balance: a fraction of receivers keep the old path (TT mask on
  GpSimd with (adj-1)*32 tiles + ScalarE Relu-bias-accum) to use idle SC/GPS.
- s1 seeding allows splitting lead receivers into local-j / partner-j ops so
  the pair h-exchange + partner hjbT matmul hide under local-j work.
- All steady-state matmuls in bf16 (fp32 is 4 cycles/row on PE).
"""

import re
import sys

sys.path.insert(0, "/opt/trn_rl_repo")

import numpy as np

import concourse.bass as bass
import concourse.bacc as bacc
import concourse.tile as tile
from concourse import mybir
from concourse import dve_ops
from concourse.dve_spec import Spec, Src0, Src1, C0, C1, relu
from concourse.dve_table_gen import dve_ver_for, free_opcode_rows
from concourse.bass_utils import run_bass_kernel_spmd
from operator import add as _add

B, N, F, H, A, T = 4, 256, 64, 256, 16, 3
NLOC = 128          # receivers per core
HT = H // 128       # h-dim tiles (2)
CHB = [(0, 64), (64, 128)]   # receiver chunks (GRU pipelining)
IB = 4              # receivers per adj broadcast group
f32 = mybir.dt.float32
bf16 = mybir.dt.bfloat16
BF16_NP = mybir.dt.np(bf16)

# ---- path assignment knobs (per 4-receiver group) -------------------------
# Every group of IB=4 consecutive receivers is either on the fused-DVE path
# ('F': adj01 tiles) or the GPS-mask + ScalarE-accum path ('S': adjm32).
# Measured: fused ~?ns/op (DVE), SC accum ~927ns, GPS mask ~531ns/recv-ht.
SC_PER_16 = 7          # of every 16 octo-groups, this many on the SC path
N_LEAD = 0             # lead split disabled (cost > bubble saved)
SB = 8                 # receivers per SC-path mask op (GPS batch)

_CACHE = {}


def _ref_fused(in0, in1, s0, s1, imm2):
    b = (np.maximum(np.nan_to_num(
            in0.astype(np.float32) + np.asarray(s0, np.float32).reshape(-1, 1),
            nan=0.0, posinf=np.inf, neginf=-np.inf), 0)
         * in1.astype(np.float32)).astype(np.float32)
    acc = b.reshape(b.shape[0], -1).sum(axis=-1, keepdims=True)
    return b, np.asarray(s1, np.float32).reshape(-1, 1) + acc


def _register_fused_op():
    if "RELU_MASK_ACC_ANT" in dve_ops._SUB_OPCODE_FOR_NAME:
        return next(o for o in dve_ops.OPS if o.name == "RELU_MASK_ACC_ANT")
    spec = Spec(
        body=relu(Src0 + C0) * Src1,
        accum=_add,
        accum_init=C1,
        reference=_ref_fused,
    )
    name = "RELU_MASK_ACC_ANT"
    ver = dve_ver_for("TRN2")
    row = max(dve_ops._SUB_OPCODE_FOR_NAME.values()) + 1
    assert row in free_opcode_rows("TRN2"), f"row {row} not free"
    dve_ops._SUB_OPCODE_FOR_NAME[name] = row
    op = dve_ops.DveOp(name, spec, subdim=False, uops_sha={ver: "?"})
    try:
        op.compile(ver)
    except ValueError as e:
        m = re.search(r"(v\d): ([0-9a-f]{16})", str(e))
        assert m, f"can't parse sha from: {e}"
        op = dve_ops.DveOp(name, spec, subdim=False,
                           uops_sha={m.group(1): m.group(2)})
    dve_ops.OPS.append(op)
    dve_ops.CUSTOM_DVE_SPECS[op.name] = op.spec
    return op


FUSED = _register_fused_op()


def _group_paths():
    """Per octo-group (8 receivers): 'S' (GPS mask + SC accum) or 'F'
    (fused DVE). 16 groups of 8 receivers; SC_PER_16 of them are 'S',
    placed at the end of each 8-group chunk half."""
    n_s = SC_PER_16
    s_set = set()
    half = n_s // 2
    for k in range(half):
        s_set.add(7 - k)        # chunk0 tail groups
    for k in range(n_s - half):
        s_set.add(15 - k)       # chunk1 tail groups
    return ['S' if g in s_set else 'F' for g in range(NLOC // SB)]


GPATHS = _group_paths()


class _WSb:
    """SBUF weight holder: W [K, M] stored as [128, (K//128)*M]."""

    def __init__(self, nc, pool, dram, K, M, name, dt=f32, eng=None):
        self.mcols = M
        self.kt = K // 128
        self.sb = pool.tile([128, self.kt * M], dt, name=name, tag=name)
        (eng or nc.sync).dma_start(out=self.sb[:], in_=dram[:])

    def __getitem__(self, sl):
        return self.sb[sl]


def build_program():
    nc = bacc.Bacc("TRN2", target_bir_lowering=False, debug=False, num_devices=8)

    # ---------------- I/O ----------------
    xT_d = nc.dram_tensor("xT", [F, N], f32, kind="ExternalInput")
    adj_d = nc.dram_tensor("adjb", [NLOC, N], bf16, kind="ExternalInput")
    deg_d = nc.dram_tensor("degr", [1, NLOC], f32, kind="ExternalInput")
    w_pre1 = nc.dram_tensor("pre_W1", [F, H], f32, kind="ExternalInput")
    w_pre2 = nc.dram_tensor("pre_W2", [128, HT * H], f32, kind="ExternalInput")
    w_m1i = nc.dram_tensor("W1i", [128, HT * H], bf16, kind="ExternalInput")
    w_m1j = nc.dram_tensor("W1j", [128, HT * H], bf16, kind="ExternalInput")
    w_m2 = nc.dram_tensor("W2m", [128, HT * H], f32, kind="ExternalInput")
    w_ih = nc.dram_tensor("Wihb", [128, HT * 3 * H], bf16, kind="ExternalInput")
    w_hh = nc.dram_tensor("Whhb", [128, HT * 3 * H], bf16, kind="ExternalInput")
    w_ro1 = nc.dram_tensor("roW1", [128, HT * H], f32, kind="ExternalInput")
    w_ro2 = nc.dram_tensor("roW2", [128, HT * A], f32, kind="ExternalInput")
    preb1_d = nc.dram_tensor("preb1c", [128, HT], f32, kind="ExternalInput")
    preb2_d = nc.dram_tensor("preb2c", [128, HT], f32, kind="ExternalInput")
    msgb1_d = nc.dram_tensor("msgb1c", [128, HT], f32, kind="ExternalInput")
    msgb2_d = nc.dram_tensor("msgb2r", [1, H], f32, kind="ExternalInput")
    brz_d = nc.dram_tensor("brzc", [128, 4], f32, kind="ExternalInput")
    bihn_d = nc.dram_tensor("bihnc", [128, HT], f32, kind="ExternalInput")
    bhhn_d = nc.dram_tensor("bhhnc", [128, HT], f32, kind="ExternalInput")
    rob1_d = nc.dram_tensor("rob1c", [128, HT], f32, kind="ExternalInput")
    rob2_d = nc.dram_tensor("rob2c", [A, 1], f32, kind="ExternalInput")
    identf_d = nc.dram_tensor("identf", [128, 128], f32, kind="ExternalInput")
    q_out = nc.dram_tensor("q_out", [A, 1], f32, kind="ExternalOutput")

    # collective bounce buffers: h exchange (bf16) for t<T-1, g exchange
    cc_in = [nc.dram_tensor(f"cc_in_{t}", [H, NLOC], bf16) for t in range(T - 1)]
    cc_out = [nc.dram_tensor(f"cc_out_{t}", [H, NLOC], bf16) for t in range(T - 1)]
    gcc_in = nc.dram_tensor("gcc_in", [1, H], f32)
    gcc_out = nc.dram_tensor("gcc_out", [1, H], f32)
    wu_in = nc.dram_tensor("wu_in", [1, 1], f32)
    wu_out = nc.dram_tensor("wu_out", [1, 1], f32)
    wu2_in = nc.dram_tensor("wu2_in", [1, 1], f32)
    wu2_out = nc.dram_tensor("wu2_out", [1, 1], f32)
    wu3_in = nc.dram_tensor("wu3_in", [1, 1], f32)
    wu3_out = nc.dram_tensor("wu3_out", [1, 1], f32)
    groups = [[0, 1], [2, 3], [4, 5], [6, 7]]

    with tile.TileContext(nc) as tc:
        import contextlib

        with contextlib.ExitStack() as ctx:
            singles = ctx.enter_context(tc.tile_pool(name="singles", bufs=1))
            work = ctx.enter_context(tc.tile_pool(name="work", bufs=3))
            eloop = ctx.enter_context(tc.tile_pool(name="eloop", bufs=16))
            psp = ctx.enter_context(tc.tile_pool(name="psp", bufs=2, space="PSUM"))
            psh = ctx.enter_context(tc.tile_pool(name="psh", bufs=1, space="PSUM"))

            # ------------- weights/constants to SBUF (critical first) -------
            xT_sb = work.tile([F, N], f32, name="xT_sb", tag="xT_sb")
            nc.sync.dma_start(out=xT_sb[:], in_=xT_d[:])
            Wpre1 = singles.tile([F, H], f32, name="Wpre1", tag="Wpre1")
            nc.sync.dma_start(out=Wpre1[:], in_=w_pre1[:])
            preb1 = singles.tile([128, HT], f32, name="preb1", tag="preb1")
            nc.sync.dma_start(out=preb1[:], in_=preb1_d[:])
            preb2 = singles.tile([128, HT], f32, name="preb2", tag="preb2")
            nc.sync.dma_start(out=preb2[:], in_=preb2_d[:])
            msgb1 = singles.tile([128, HT], f32, name="msgb1", tag="msgb1")
            nc.sync.dma_start(out=msgb1[:], in_=msgb1_d[:])
            W_pre2 = _WSb(nc, singles, w_pre2[:], H, H, "Wpre2", f32)
            W_m1j = _WSb(nc, singles, w_m1j[:], H, H, "Wm1j", bf16)
            W_m1i = _WSb(nc, singles, w_m1i[:], H, H, "Wm1i", bf16)
            W_hh = _WSb(nc, singles, w_hh[:], H, 3 * H, "Whh", bf16, eng=nc.gpsimd)
            W_m2 = _WSb(nc, singles, w_m2[:], H, H, "Wm2", f32, eng=nc.gpsimd)
            W_ih = _WSb(nc, singles, w_ih[:], H, 3 * H, "Wih", bf16, eng=nc.gpsimd)

            def _load(shape, dram, name, eng=nc.sync, dt=f32):
                t_ = singles.tile(list(shape), dt, name=name, tag=name)
                eng.dma_start(out=t_[:], in_=dram[:])
                return t_

            msgb2 = _load([1, H], msgb2_d, "msgb2")
            brz = _load([128, 4], brz_d, "brz")
            bihn = _load([128, HT], bihn_d, "bihn")
            bhhn = _load([128, HT], bhhn_d, "bhhn")
            deg_row = _load([1, NLOC], deg_d, "degr")
            W_ro1 = _WSb(nc, singles, w_ro1[:], H, H, "Wro1", f32, eng=nc.gpsimd)
            W_ro2 = _WSb(nc, singles, w_ro2[:], H, A, "Wro2", f32, eng=nc.gpsimd)
            rob1 = _load([128, HT], rob1_d, "rob1", eng=nc.gpsimd)
            rob2 = singles.tile([A, 1], f32, name="rob2", tag="rob2")
            nc.gpsimd.dma_start(out=rob2[:], in_=rob2_d[:])
            identf = singles.tile([128, 128], f32, name="identf", tag="identf")
            nc.gpsimd.dma_start(out=identf[:], in_=identf_d[:])
            ones11 = singles.tile([1, 1], f32, name="ones11", tag="ones11")
            nc.vector.memset(ones11[:], 1.0)

            # absolute receiver -> path ('F' fused / 'S' SC)
            rpath = []
            for g, p in enumerate(GPATHS):
                rpath += [p] * SB
            lead = [i for i in range(N_LEAD) if rpath[i] == 'F']

            # octo groups (8 receivers per broadcast tile)
            tt_groups = [list(range(k, k + SB)) for k in range(0, NLOC, SB)]

            # hold the GPS DMA ring until the critical weights are in
            gate = singles.tile([1, 1], bf16, name="gate", tag="gate")
            nc.gpsimd.tensor_copy(gate[:], W_m1i[0:1, 0:1])
            adj_bcg = {}
            for grp in tt_groups:
                r0, gl = grp[0], len(grp)
                tl = singles.tile([128, gl * N], bf16, name=f"adjg{r0}", tag=f"adjg{r0}")
                bc_in = bass.AP(
                    tensor=adj_d, offset=r0 * N,
                    ap=[[0, 128], [N, gl], [1, N]],
                )
                nc.gpsimd.dma_start(out=tl[:], in_=bc_in)
                adj_bcg[r0] = tl

            # warm up the collective engine during the startup DMA window
            nc.gpsimd.collective_compute(
                "AllReduce", mybir.AluOpType.add, replica_groups=groups,
                ins=[wu_in[:]], outs=[wu_out[:]])
            nc.gpsimd.collective_compute(
                "AllReduce", mybir.AluOpType.add, replica_groups=groups,
                ins=[wu2_in[:]], outs=[wu2_out[:]])
            # prime the sigmoid/tanh activation table while ScalarE is idle
            actwu = singles.tile([1, 2], f32, name="actwu", tag="actwu")
            nc.vector.memset(actwu[:], 0.0)
            nc.scalar.activation(actwu[:, 0:1], actwu[:, 1:2], mybir.ActivationFunctionType.Sigmoid)
            nc.scalar.activation(actwu[:, 0:1], actwu[:, 1:2], mybir.ActivationFunctionType.Tanh)

            # ---------------- preprocess: h0 (f32 for accuracy) -------------
            p1 = [work.tile([128, N], f32, name=f"p1_{ht}", tag=f"p1_{ht}") for ht in range(HT)]
            for ht in range(HT):
                ps = psp.tile([128, 512], f32, name="ps", tag="ps")
                nc.tensor.matmul(ps[:, 0:N], Wpre1[:, ht * 128:(ht + 1) * 128], xT_sb[:], start=True, stop=True)
                nc.scalar.activation(p1[ht][:], ps[:, 0:N], mybir.ActivationFunctionType.Relu, bias=preb1[:, ht:ht + 1])
            hTf = [singles.tile([128, N], f32, name=f"hTf{ht}", tag=f"hTf{ht}") for ht in range(HT)]
            hTb = [singles.tile([128, N], bf16, name=f"hTbb{ht}", tag=f"hTbb{ht}") for ht in range(HT)]
            for ht in range(HT):
                ps = psp.tile([128, 512], f32, name="ps", tag="ps")
                for kt in range(HT):
                    nc.tensor.matmul(ps[:, 0:N], W_pre2[:, kt * H + ht * 128: kt * H + (ht + 1) * 128], p1[kt][:], start=(kt == 0), stop=(kt == HT - 1))
                nc.scalar.activation(hTf[ht][:], ps[:, 0:N], mybir.ActivationFunctionType.Identity, bias=preb2[:, ht:ht + 1])
                nc.vector.tensor_copy(hTb[ht][:], hTf[ht][:])

            # ---------------- message passing iterations ----------------
            for t in range(T):
                # hiT = (h_loc @ W1_i).T [h, i] f32 (bias-free; bf16 matmul)
                hiTf = [work.tile([128, NLOC], f32, name=f"hiTf{ht}", tag=f"hiTf{ht}") for ht in range(HT)]
                for ht in range(HT):
                    ps = psp.tile([128, 512], f32, name="ps", tag="ps")
                    for kt in range(HT):
                        nc.tensor.matmul(ps[:, 0:NLOC], W_m1i[:, kt * H + ht * 128: kt * H + (ht + 1) * 128], hTb[kt][:, 0:NLOC], start=(kt == 0), stop=(kt == HT - 1))
                    nc.scalar.activation(hiTf[ht][:], ps[:, 0:NLOC], mybir.ActivationFunctionType.Identity)
                # hjbT = (h @ W1_j + b1).T [h, j] bf16; local half only here.
                # Partner half is emitted after the leads-local fused ops so
                # the DVE queue is not blocked behind the exchange subtracts.
                hjbT = [work.tile([128, N], bf16, name=f"hjbT{ht}", tag=f"hjbT{ht}") for ht in range(HT)]
                psj = []
                for ht in range(HT):
                    ps = psp.tile([128, 512], f32, name="ps", tag="ps")
                    psj.append(ps)
                    for kt in range(HT):
                        nc.tensor.matmul(ps[:, 0:NLOC], W_m1j[:, kt * H + ht * 128: kt * H + (ht + 1) * 128], hTb[kt][:, 0:NLOC], start=(kt == 0), stop=(kt == HT - 1))
                    nc.scalar.activation(hjbT[ht][:, 0:NLOC], ps[:, 0:NLOC], mybir.ActivationFunctionType.Identity, bias=msgb1[:, ht:ht + 1])

                ps_rz = psh.tile([128, 512], f32, name="ps_rz", tag="ps_rz")
                ps_gh = psh.tile([128, 512], f32, name="ps_gh", tag="ps_gh")

                aggT = [work.tile([128, NLOC], f32, name=f"aggT{ht}", tag=f"aggT{ht}") for ht in range(HT)]
                rz_sb = work.tile([128, 512], f32, name="rz_sb", tag="rz_sb")

                def emit_fused(i, split=None):
                    """Fused e-ops for receiver i (both h-tiles). split:
                    None = whole row; 'local'/'partner' = j-half with accum
                    chaining via the s1 seed."""
                    for ht in range(HT):
                        scr = eloop.tile([128, N], bf16, name="scr", tag="scr")
                        a01 = adj_bcg[(i // SB) * SB]
                        aoff = (i % SB) * N
                        if split == 'local':
                            nc.vector._custom_dve(
                                FUSED, out=scr[:, 0:NLOC],
                                in0=hjbT[ht][:, 0:NLOC],
                                in1=a01[:, aoff:aoff + NLOC],
                                s0=hiTf[ht][:, i:i + 1], s1=0.0,
                                accum_out=aggT[ht][:, i:i + 1])
                        elif split == 'partner':
                            nc.vector._custom_dve(
                                FUSED, out=scr[:, NLOC:N],
                                in0=hjbT[ht][:, NLOC:N],
                                in1=a01[:, aoff + NLOC:aoff + N],
                                s0=hiTf[ht][:, i:i + 1],
                                s1=aggT[ht][:, i:i + 1],
                                accum_out=aggT[ht][:, i:i + 1])
                        else:
                            nc.vector._custom_dve(
                                FUSED, out=scr[:],
                                in0=hjbT[ht][:],
                                in1=a01[:, aoff:aoff + N],
                                s0=hiTf[ht][:, i:i + 1], s1=0.0,
                                accum_out=aggT[ht][:, i:i + 1])

                def emit_sc_group(grp):
                    """GPS mask + ScalarE relu-bias-accum for 8 receivers."""
                    r0 = grp[0]
                    for ht in range(HT):
                        hjb_rep = bass.AP(
                            tensor=hjbT[ht].tensor, offset=hjbT[ht].offset,
                            ap=[hjbT[ht].ap[0], [0, len(grp)], [1, N]])
                        w = eloop.tile([128, SB * N], bf16, name="w", tag="w")
                        nc.vector.tensor_tensor(
                            out=w[:, 0:len(grp) * N], in0=hjb_rep,
                            in1=adj_bcg[r0][:], op=mybir.AluOpType.add)
                        for k, i in enumerate(grp):
                            scr = eloop.tile([128, N], bf16, name="scs", tag="scs")
                            nc.scalar.activation(
                                scr[:], w[:, k * N:(k + 1) * N],
                                mybir.ActivationFunctionType.Relu,
                                bias=hiTf[ht][:, i:i + 1],
                                accum_out=aggT[ht][:, i:i + 1])

                def emit_gru(c):
                    C0_, C1_ = CHB[c]
                    CWc = C1_ - C0_
                    # msgT = W2m.T @ agg + deg * b2  (bf16 matmuls)
                    ps_m = psp.tile([128, 512], f32, name="ps", tag="ps")
                    for ht in range(HT):
                        for kt in range(HT):
                            nc.tensor.matmul(ps_m[:, ht * CWc:(ht + 1) * CWc], W_m2[:, kt * H + ht * 128: kt * H + (ht + 1) * 128], aggT[kt][:, C0_:C1_], start=(kt == 0), stop=False)
                        nc.tensor.matmul(ps_m[:, ht * CWc:(ht + 1) * CWc], msgb2[0:1, ht * 128:(ht + 1) * 128], deg_row[0:1, C0_:C1_], start=False, stop=True)
                    msgTb = work.tile([128, 2 * 64], bf16, name="msgTb", tag="msgTb")
                    for ht in range(HT):
                        nc.vector.tensor_copy(msgTb[:, ht * CWc:(ht + 1) * CWc], ps_m[:, ht * CWc:(ht + 1) * CWc])
                    # gate matmuls (bf16): Wih then Whh per psum region
                    for mt in range(4):
                        for kt in range(HT):
                            nc.tensor.matmul(ps_rz[:, mt * 128 + C0_: mt * 128 + C1_], W_ih[:, kt * 768 + mt * 128: kt * 768 + (mt + 1) * 128], msgTb[:, kt * CWc:(kt + 1) * CWc], start=(kt == 0), stop=False)
                        for kt in range(HT):
                            nc.tensor.matmul(ps_rz[:, mt * 128 + C0_: mt * 128 + C1_], W_hh[:, kt * 768 + mt * 128: kt * 768 + (mt + 1) * 128], hTb[kt][:, C0_:C1_], start=False, stop=(kt == HT - 1))
                    for ht in range(HT):
                        for kt in range(HT):
                            nc.tensor.matmul(ps_gh[:, ht * 128 + C0_: ht * 128 + C1_], W_ih[:, kt * 768 + (4 + ht) * 128: kt * 768 + (5 + ht) * 128], msgTb[:, kt * CWc:(kt + 1) * CWc], start=(kt == 0), stop=(kt == HT - 1))
                        for kt in range(HT):
                            nc.tensor.matmul(ps_gh[:, 256 + ht * 128 + C0_: 256 + ht * 128 + C1_], W_hh[:, kt * 768 + (4 + ht) * 128: kt * 768 + (5 + ht) * 128], hTb[kt][:, C0_:C1_], start=(kt == 0), stop=(kt == HT - 1))
                    for mt in range(4):
                        nc.scalar.activation(rz_sb[:, mt * 128 + C0_: mt * 128 + C1_], ps_rz[:, mt * 128 + C0_: mt * 128 + C1_], mybir.ActivationFunctionType.Sigmoid, bias=brz[:, mt:mt + 1])
                    for ht in range(HT):
                        # rhn = (gh_n + bhhn) * r
                        rhn = work.tile([128, 80], f32, name="rhn", tag="rhn")[:, 0:CWc]
                        nc.vector.scalar_tensor_tensor(
                            out=rhn, in0=ps_gh[:, 256 + ht * 128 + C0_: 256 + ht * 128 + C1_],
                            scalar=bhhn[:, ht:ht + 1], in1=rz_sb[:, ht * 128 + C0_: ht * 128 + C1_],
                            op0=mybir.AluOpType.add, op1=mybir.AluOpType.mult)
                        nsum = work.tile([128, 80], f32, name="nsum", tag="nsum")[:, 0:CWc]
                        nc.vector.scalar_tensor_tensor(
                            out=nsum, in0=ps_gh[:, ht * 128 + C0_: ht * 128 + C1_],
                            scalar=bihn[:, ht:ht + 1], in1=rhn,
                            op0=mybir.AluOpType.add, op1=mybir.AluOpType.add)
                        n_t = work.tile([128, 80], f32, name="n_t", tag="n_t")[:, 0:CWc]
                        nc.scalar.activation(n_t, nsum, mybir.ActivationFunctionType.Tanh)
                        hmn = work.tile([128, 80], f32, name="hmn", tag="hmn")[:, 0:CWc]
                        nc.gpsimd.tensor_sub(hmn, hTf[ht][:, C0_:C1_], n_t)
                        zh = work.tile([128, 80], f32, name="zh", tag="zh")[:, 0:CWc]
                        nc.gpsimd.tensor_mul(zh, rz_sb[:, 256 + ht * 128 + C0_: 256 + ht * 128 + C1_], hmn)
                        nc.vector.tensor_add(hTf[ht][:, C0_:C1_], n_t, zh)
                        nc.scalar.activation(hTb[ht][:, C0_:C1_], hTf[ht][:, C0_:C1_], mybir.ActivationFunctionType.Identity)
                        if t < T - 1:
                            nc.sync.dma_start(out=cc_in[t][ht * 128:(ht + 1) * 128, C0_:C1_], in_=hTb[ht][:, C0_:C1_])

                # ---- emission order ----
                # 1. leads-local fused (DVE busy while the exchange lands)
                lead_set = set(lead) if t > 0 else set()
                for i in sorted(lead_set):
                    emit_fused(i, split='local')
                # 2. exchange completion: partner h = sum - local (bf16)
                if t > 0:
                    for ht in range(HT):
                        nc.vector.tensor_sub(hTb[ht][:, NLOC:N], rs_prev[ht][:], hTb[ht][:, 0:NLOC])
                # 3. hjbT partner half
                for ht in range(HT):
                    ps = psj[ht]
                    for kt in range(HT):
                        nc.tensor.matmul(ps[:, NLOC:N], W_m1j[:, kt * H + ht * 128: kt * H + (ht + 1) * 128], hTb[kt][:, NLOC:N], start=(kt == 0), stop=(kt == HT - 1))
                    nc.scalar.activation(hjbT[ht][:, NLOC:N], ps[:, NLOC:N], mybir.ActivationFunctionType.Identity, bias=msgb1[:, ht:ht + 1])
                # 4. leads-partner (seeded accum)
                for i in sorted(lead_set):
                    emit_fused(i, split='partner')
                # 5. all S-group masks early (GPS queue) + their SC accums
                for grp in tt_groups:
                    if rpath[grp[0]] == 'S':
                        emit_sc_group(grp)
                # 6. fused receivers chunk0, GRU(0), chunk1, GRU(1)
                for grp in tt_groups:
                    r0 = grp[0]
                    if r0 >= CHB[0][1] or rpath[r0] == 'S':
                        continue
                    for i in grp:
                        if i not in lead_set:
                            emit_fused(i)
                emit_gru(0)
                for grp in tt_groups:
                    r0 = grp[0]
                    if r0 < CHB[0][1] or rpath[r0] == 'S':
                        continue
                    for i in grp:
                        if i not in lead_set:
                            emit_fused(i)
                emit_gru(1)

                if t < T - 1:
                    nc.gpsimd.collective_compute(
                        "AllReduce", mybir.AluOpType.add, replica_groups=groups,
                        ins=[cc_in[t][:]], outs=[cc_out[t][:]])
                    if t == T - 2:
                        nc.gpsimd.collective_compute(
                            "AllReduce", mybir.AluOpType.add, replica_groups=groups,
                            ins=[wu3_in[:]], outs=[wu3_out[:]])
                    rs_prev = []
                    for ht in range(HT):
                        rs = work.tile([128, NLOC], bf16, name="rs", tag="rs")
                        nc.sync.dma_start(out=rs[:], in_=cc_out[t][ht * 128:(ht + 1) * 128, :])
                        rs_prev.append(rs)

            # ---------------- readout ----------------
            # g as a [1, 256] row so the collective bounce DMAs are single
            # descriptors (partition-major [128,1] DMAs cost ~10us each).
            gT = [work.tile([128, 1], f32, name=f"gT{ht}", tag=f"gT{ht}") for ht in range(HT)]
            grow = work.tile([1, 2 * 128], f32, name="grow", tag="grow")
            ps_g = psp.tile([128, 512], f32, name="ps", tag="ps")
            for ht in range(HT):
                nc.vector.reduce_sum(gT[ht][:], hTf[ht][:, 0:NLOC], axis=mybir.AxisListType.X)
                # transpose [128,1] -> [1,128] via identity-rhs matmul
                nc.tensor.matmul(ps_g[0:1, ht * 128:(ht + 1) * 128], gT[ht][:], identf[:], start=True, stop=True)
            nc.vector.tensor_copy(grow[:], ps_g[0:1, 0:256])
            nc.sync.dma_start(out=gcc_in[:], in_=grow[:])
            nc.gpsimd.collective_compute(
                "AllReduce", mybir.AluOpType.add, replica_groups=groups,
                ins=[gcc_in[:]], outs=[gcc_out[:]])
            gsrow = work.tile([1, 2 * 128], f32, name="gsrow", tag="gsrow")
            nc.sync.dma_start(out=gsrow[:], in_=gcc_out[:])
            gs = [work.tile([128, 1], f32, name=f"gs{ht}", tag=f"gs{ht}") for ht in range(HT)]
            ps_g2 = psp.tile([128, 512], f32, name="ps", tag="ps")
            for ht in range(HT):
                # [1,128] row -> [128,1] column via K=1 outer with ones
                nc.tensor.matmul(ps_g2[0:128, ht:ht + 1], gsrow[0:1, ht * 128:(ht + 1) * 128], ones11[0:1, 0:1], start=True, stop=True)
                nc.vector.tensor_copy(gs[ht][:], ps_g2[:, ht:ht + 1])
            y1 = [work.tile([128, 1], f32, name=f"y1{ht}", tag=f"y1{ht}") for ht in range(HT)]
            for ht in range(HT):
                ps = psp.tile([128, 512], f32, name="ps", tag="ps")
                for kt in range(HT):
                    nc.tensor.matmul(ps[:, 0:1], W_ro1[:, kt * H + ht * 128: kt * H + (ht + 1) * 128], gs[kt][:], start=(kt == 0), stop=(kt == HT - 1))
                nc.scalar.activation(y1[ht][:], ps[:, 0:1], mybir.ActivationFunctionType.Relu, bias=rob1[:, ht:ht + 1])
            ps_q = psp.tile([128, 512], f32, name="ps", tag="ps")
            for kt in range(HT):
                nc.tensor.matmul(ps_q[0:A, 0:1], W_ro2[:, kt * A:(kt + 1) * A], y1[kt][:], start=(kt == 0), stop=(kt == HT - 1))
            q_sb = work.tile([A, 1], f32, name="q_sb", tag="q_sb")
            nc.scalar.activation(q_sb[:], ps_q[0:A, 0:1], mybir.ActivationFunctionType.Identity, bias=rob2[:])
            nc.sync.dma_start(out=q_out[:], in_=q_sb[:])

    nc.compile()
    return nc


def _in_maps(inputs):
    nf = np.asarray(inputs["node_features"], np.float32)
    adj = np.asarray(inputs["adjacency"])
    msg_W1 = np.asarray(inputs["msg_W1"], np.float32)
    gbih = np.asarray(inputs["gru_bih"], np.float32)
    gbhh = np.asarray(inputs["gru_bhh"], np.float32)

    def cols(v, nt):  # [nt*128] -> [128, nt] partition-major columns
        return np.ascontiguousarray(np.asarray(v, np.float32).reshape(nt, 128).T)

    def wsb(w, dt=np.float32):  # [K, M] -> [128, (K//128)*M]
        w = np.asarray(w, np.float32)
        K, M = w.shape
        return np.ascontiguousarray(
            np.concatenate([w[k * 128:(k + 1) * 128] for k in range(K // 128)], axis=1)
        ).astype(dt)

    # per-receiver mask form: fused groups get 0/1, SC groups get (a-1)*32
    rform = np.zeros(NLOC, np.int32)  # 0 -> adj01, 1 -> adjm32
    for g, p in enumerate(GPATHS):
        if p == 'S':
            rform[g * SB:(g + 1) * SB] = 1

    shared = {
        "pre_W1": np.asarray(inputs["pre_W1"], np.float32),
        "pre_W2": wsb(inputs["pre_W2"]),
        "W1i": wsb(msg_W1[:H], BF16_NP),
        "W1j": wsb(msg_W1[H:], BF16_NP),
        "W2m": wsb(inputs["msg_W2"]),
        "Wihb": wsb(inputs["gru_Wih"], BF16_NP),
        "Whhb": wsb(inputs["gru_Whh"], BF16_NP),
        "roW1": wsb(inputs["ro_W1"]),
        "roW2": wsb(inputs["ro_W2"]),
        "preb1c": cols(inputs["pre_b1"], HT),
        "preb2c": cols(inputs["pre_b2"], HT),
        "msgb1c": cols(inputs["msg_b1"], HT),
        "msgb2r": np.asarray(inputs["msg_b2"], np.float32)[None, :],
        "brzc": cols((gbih + gbhh)[: 2 * H], 4),
        "bihnc": cols(gbih[2 * H:], HT),
        "bhhnc": cols(gbhh[2 * H:], HT),
        "rob1c": cols(inputs["ro_b1"], HT),
        "rob2c": np.asarray(inputs["ro_b2"], np.float32)[:, None],
        "identf": np.eye(128, dtype=np.float32),
    }
    maps = []
    for c in range(8):
        b, half = c // 2, c % 2
        lo, hi = half * NLOC, (half + 1) * NLOC
        perm = np.r_[lo:hi, 0:lo, hi:N]
        m = dict(shared)
        m["xT"] = np.ascontiguousarray(nf[b].T[:, perm])
        a = adj[b, lo:hi][:, perm].astype(np.float32)
        adjmix = np.where(rform[:, None] == 1, (a - 1) * 32.0, a)
        m["adjb"] = adjmix.astype(BF16_NP)
        m["degr"] = adj[b, lo:hi].sum(axis=1).astype(np.float32)[None, :]
        maps.append(m)
    return maps


def kernel(**inputs) -> np.ndarray:
    if "nc" not in _CACHE:
        _CACHE["nc"] = build_program()
    nc = _CACHE["nc"]
    maps = _in_maps(inputs)
    res = run_bass_kernel_spmd(nc, maps, list(range(8))).results
    q = np.stack([res[2 * b]["q_out"][:, 0] for b in range(B)]).astype(np.float32)
    return q


# revision 4
# speedup vs baseline: 1.0187x; 1.0187x over previous
"""Trainium2 Bass kernel for nn_MessagePassingNN (gnn_message_passing).

B, N, F, H, A, T = 4, 256, 64, 256, 16, 3

Sharding: 8 cores = (batch b = c//2, receiver-half c%2). Node indexing is
core-relative ([my 128 | partner 128]); host permutes inputs per core.

v4 design (from HW trace analysis; v2 two-op e-loop baseline was 405-472us,
this measures ~340-350us on low-skew runs):
- e-loop core: RUNTIME-REGISTERED custom DVE op RELU_MASK_ACC_ANT
    accum_out = s1 + sum_j relu(hjbT[h,j] + hiT[h,i]) * adj01[i,j]
  ONE DVE instruction per (receiver, h-tile) replaces the v2 TT-mask +
  STT/ScalarE pair. adj rides in1 as an fp8 0/1 broadcast tile; the
  throwaway out tile is fp8 (SBUF traffic 6B -> 4B per element).
- Hybrid balance: 7/16 of receivers keep a two-op path (DVE TT mask with
  (adj-1)*32 bf16 tiles at 8 receivers per op + ScalarE Relu-bias-accum)
  to load the otherwise idle ScalarE. GPSIMD does NO tensor work in the
  e-loop: measured, a GpSimd tensor op starves concurrent DVE custom ops
  ~9x (SBUF arbitration), regardless of which tiles either reads.
- All per-iteration matmuls bf16 (fp32 is 4 PE cycles/row); msg_W2/agg and
  the preprocess stay f32 for accuracy (bf16 W2 alone costs 1.3e-2 rel err).
- h-exchange: one bf16 pairwise AllReduce per iteration (sum - local
  reconstructs the partner); no startup warmup collectives - they queue
  ahead of the real exchanges on the CC stream and add latency when the
  cross-core launch skew is large (observed 20-90us NEFF start barrier).
- readout bounces g through a [1, 256] row (PE transpose) so the collective
  DMAs are single descriptors; partition-major [128,1] DMAs cost ~10us.
"""

import re
import sys

sys.path.insert(0, "/opt/trn_rl_repo")

import numpy as np

import concourse.bass as bass
import concourse.bacc as bacc
import concourse.tile as tile
from concourse import mybir
from concourse import dve_ops
from concourse.dve_spec import Spec, Src0, Src1, C0, C1, relu
from concourse.dve_table_gen import dve_ver_for, free_opcode_rows
from concourse.bass_utils import run_bass_kernel_spmd
from operator import add as _add

B, N, F, H, A, T = 4, 256, 64, 256, 16, 3
NLOC = 128          # receivers per core
HT = H // 128       # h-dim tiles (2)
CHB = [(0, 64), (64, 128)]   # receiver chunks (GRU pipelining)
IB = 4              # receivers per adj broadcast group
f32 = mybir.dt.float32
bf16 = mybir.dt.bfloat16
fp8 = mybir.dt.float8e4
BF16_NP = mybir.dt.np(bf16)
FP8_NP = mybir.dt.np(fp8)

# ---- path assignment knobs (per 4-receiver group) -------------------------
# Every group of IB=4 consecutive receivers is either on the fused-DVE path
# ('F': adj01 tiles) or the GPS-mask + ScalarE-accum path ('S': adjm32).
# Measured: fused ~?ns/op (DVE), SC accum ~927ns, GPS mask ~531ns/recv-ht.
SC_PER_16 = 8          # of every 16 octo-groups, this many on the SC path
N_LEAD = 0             # lead split disabled (cost > bubble saved)
SB = 8                 # receivers per SC-path mask op (GPS batch)

_CACHE = {}


def _ref_fused(in0, in1, s0, s1, imm2):
    b = (np.maximum(np.nan_to_num(
            in0.astype(np.float32) + np.asarray(s0, np.float32).reshape(-1, 1),
            nan=0.0, posinf=np.inf, neginf=-np.inf), 0)
         * in1.astype(np.float32)).astype(np.float32)
    acc = b.reshape(b.shape[0], -1).sum(axis=-1, keepdims=True)
    return b, np.asarray(s1, np.float32).reshape(-1, 1) + acc


def _register_fused_op():
    if "RELU_MASK_ACC_ANT" in dve_ops._SUB_OPCODE_FOR_NAME:
        return next(o for o in dve_ops.OPS if o.name == "RELU_MASK_ACC_ANT")
    spec = Spec(
        body=relu(Src0 + C0) * Src1,
        accum=_add,
        accum_init=C1,
        reference=_ref_fused,
    )
    name = "RELU_MASK_ACC_ANT"
    ver = dve_ver_for("TRN2")
    row = max(dve_ops._SUB_OPCODE_FOR_NAME.values()) + 1
    assert row in free_opcode_rows("TRN2"), f"row {row} not free"
    dve_ops._SUB_OPCODE_FOR_NAME[name] = row
    op = dve_ops.DveOp(name, spec, subdim=False, uops_sha={ver: "?"})
    try:
        op.compile(ver)
    except ValueError as e:
        m = re.search(r"(v\d): ([0-9a-f]{16})", str(e))
        assert m, f"can't parse sha from: {e}"
        op = dve_ops.DveOp(name, spec, subdim=False,
                           uops_sha={m.group(1): m.group(2)})
    dve_ops.OPS.append(op)
    dve_ops.CUSTOM_DVE_SPECS[op.name] = op.spec
    return op


FUSED = _register_fused_op()


def _group_paths():
    """Per octo-group (8 receivers): 'S' (GPS mask + SC accum) or 'F'
    (fused DVE). 16 groups of 8 receivers; SC_PER_16 of them are 'S',
    placed at the end of each 8-group chunk half."""
    n_s = SC_PER_16
    s_set = set()
    half = n_s // 2
    for k in range(half):
        s_set.add(7 - k)        # chunk0 tail groups
    for k in range(n_s - half):
        s_set.add(15 - k)       # chunk1 tail groups
    return ['S' if g in s_set else 'F' for g in range(NLOC // SB)]


GPATHS = _group_paths()


class _WSb:
    """SBUF weight holder: W [K, M] stored as [128, (K//128)*M]."""

    def __init__(self, nc, pool, dram, K, M, name, dt=f32, eng=None):
        self.mcols = M
        self.kt = K // 128
        self.sb = pool.tile([128, self.kt * M], dt, name=name, tag=name)
        (eng or nc.sync).dma_start(out=self.sb[:], in_=dram[:])

    def __getitem__(self, sl):
        return self.sb[sl]


def build_program():
    nc = bacc.Bacc("TRN2", target_bir_lowering=False, debug=False, num_devices=8)

    # ---------------- I/O ----------------
    xT_d = nc.dram_tensor("xT", [F, N], f32, kind="ExternalInput")
    adj_d = nc.dram_tensor("adjb", [NLOC, N], bf16, kind="ExternalInput")
    adj8_d = nc.dram_tensor("adj8", [NLOC, N], fp8, kind="ExternalInput")
    deg_d = nc.dram_tensor("degr", [1, NLOC], f32, kind="ExternalInput")
    w_pre1 = nc.dram_tensor("pre_W1", [F, H], f32, kind="ExternalInput")
    w_pre2 = nc.dram_tensor("pre_W2", [128, HT * H], f32, kind="ExternalInput")
    w_m1i = nc.dram_tensor("W1i", [128, HT * H], bf16, kind="ExternalInput")
    w_m1j = nc.dram_tensor("W1j", [128, HT * H], bf16, kind="ExternalInput")
    w_m2 = nc.dram_tensor("W2m", [128, HT * H], f32, kind="ExternalInput")
    w_ih = nc.dram_tensor("Wihb", [128, HT * 3 * H], bf16, kind="ExternalInput")
    w_hh = nc.dram_tensor("Whhb", [128, HT * 3 * H], bf16, kind="ExternalInput")
    w_ro1 = nc.dram_tensor("roW1", [128, HT * H], f32, kind="ExternalInput")
    w_ro2 = nc.dram_tensor("roW2", [128, HT * A], f32, kind="ExternalInput")
    preb1_d = nc.dram_tensor("preb1c", [128, HT], f32, kind="ExternalInput")
    preb2_d = nc.dram_tensor("preb2c", [128, HT], f32, kind="ExternalInput")
    msgb1_d = nc.dram_tensor("msgb1c", [128, HT], f32, kind="ExternalInput")
    msgb2_d = nc.dram_tensor("msgb2r", [1, H], f32, kind="ExternalInput")
    brz_d = nc.dram_tensor("brzc", [128, 4], f32, kind="ExternalInput")
    bihn_d = nc.dram_tensor("bihnc", [128, HT], f32, kind="ExternalInput")
    bhhn_d = nc.dram_tensor("bhhnc", [128, HT], f32, kind="ExternalInput")
    rob1_d = nc.dram_tensor("rob1c", [128, HT], f32, kind="ExternalInput")
    rob2_d = nc.dram_tensor("rob2c", [A, 1], f32, kind="ExternalInput")
    identf_d = nc.dram_tensor("identf", [128, 128], f32, kind="ExternalInput")
    q_out = nc.dram_tensor("q_out", [A, 1], f32, kind="ExternalOutput")

    # collective bounce buffers: h exchange (bf16) for t<T-1, g exchange
    cc_in = [nc.dram_tensor(f"cc_in_{t}", [H, NLOC], bf16) for t in range(T - 1)]
    cc_out = [nc.dram_tensor(f"cc_out_{t}", [H, NLOC], bf16) for t in range(T - 1)]
    gcc_in = nc.dram_tensor("gcc_in", [1, H], f32)
    gcc_out = nc.dram_tensor("gcc_out", [1, H], f32)
    wu_in = nc.dram_tensor("wu_in", [1, 1], f32)
    wu_out = nc.dram_tensor("wu_out", [1, 1], f32)
    wu2_in = nc.dram_tensor("wu2_in", [1, 1], f32)
    wu2_out = nc.dram_tensor("wu2_out", [1, 1], f32)
    wu3_in = nc.dram_tensor("wu3_in", [1, 1], f32)
    wu3_out = nc.dram_tensor("wu3_out", [1, 1], f32)
    groups = [[0, 1], [2, 3], [4, 5], [6, 7]]

    with tile.TileContext(nc) as tc:
        import contextlib

        with contextlib.ExitStack() as ctx:
            singles = ctx.enter_context(tc.tile_pool(name="singles", bufs=1))
            work = ctx.enter_context(tc.tile_pool(name="work", bufs=3))
            eloop = ctx.enter_context(tc.tile_pool(name="eloop", bufs=16))
            psp = ctx.enter_context(tc.tile_pool(name="psp", bufs=2, space="PSUM"))
            psh = ctx.enter_context(tc.tile_pool(name="psh", bufs=1, space="PSUM"))

            # ------------- weights/constants to SBUF (critical first) -------
            xT_sb = work.tile([F, N], f32, name="xT_sb", tag="xT_sb")
            nc.sync.dma_start(out=xT_sb[:], in_=xT_d[:])
            Wpre1 = singles.tile([F, H], f32, name="Wpre1", tag="Wpre1")
            nc.sync.dma_start(out=Wpre1[:], in_=w_pre1[:])
            preb1 = singles.tile([128, HT], f32, name="preb1", tag="preb1")
            nc.sync.dma_start(out=preb1[:], in_=preb1_d[:])
            preb2 = singles.tile([128, HT], f32, name="preb2", tag="preb2")
            nc.sync.dma_start(out=preb2[:], in_=preb2_d[:])
            msgb1 = singles.tile([128, HT], f32, name="msgb1", tag="msgb1")
            nc.sync.dma_start(out=msgb1[:], in_=msgb1_d[:])
            W_pre2 = _WSb(nc, singles, w_pre2[:], H, H, "Wpre2", f32)
            W_m1j = _WSb(nc, singles, w_m1j[:], H, H, "Wm1j", bf16)
            W_m1i = _WSb(nc, singles, w_m1i[:], H, H, "Wm1i", bf16)
            W_hh = _WSb(nc, singles, w_hh[:], H, 3 * H, "Whh", bf16, eng=nc.gpsimd)
            W_m2 = _WSb(nc, singles, w_m2[:], H, H, "Wm2", f32, eng=nc.gpsimd)
            W_ih = _WSb(nc, singles, w_ih[:], H, 3 * H, "Wih", bf16, eng=nc.gpsimd)

            def _load(shape, dram, name, eng=nc.sync, dt=f32):
                t_ = singles.tile(list(shape), dt, name=name, tag=name)
                eng.dma_start(out=t_[:], in_=dram[:])
                return t_

            msgb2 = _load([1, H], msgb2_d, "msgb2")
            brz = _load([128, 4], brz_d, "brz")
            bihn = _load([128, HT], bihn_d, "bihn")
            bhhn = _load([128, HT], bhhn_d, "bhhn")
            deg_row = _load([1, NLOC], deg_d, "degr")
            W_ro1 = _WSb(nc, singles, w_ro1[:], H, H, "Wro1", f32, eng=nc.gpsimd)
            W_ro2 = _WSb(nc, singles, w_ro2[:], H, A, "Wro2", f32, eng=nc.gpsimd)
            rob1 = _load([128, HT], rob1_d, "rob1", eng=nc.gpsimd)
            rob2 = singles.tile([A, 1], f32, name="rob2", tag="rob2")
            nc.gpsimd.dma_start(out=rob2[:], in_=rob2_d[:])
            identf = singles.tile([128, 128], f32, name="identf", tag="identf")
            nc.gpsimd.dma_start(out=identf[:], in_=identf_d[:])
            ones11 = singles.tile([1, 1], f32, name="ones11", tag="ones11")
            nc.vector.memset(ones11[:], 1.0)

            # absolute receiver -> path ('F' fused / 'S' SC)
            rpath = []
            for g, p in enumerate(GPATHS):
                rpath += [p] * SB
            lead = [i for i in range(N_LEAD) if rpath[i] == 'F']

            # octo groups (8 receivers per broadcast tile)
            tt_groups = [list(range(k, k + SB)) for k in range(0, NLOC, SB)]

            # hold the GPS DMA ring until the critical weights are in
            gate = singles.tile([1, 1], bf16, name="gate", tag="gate")
            nc.gpsimd.tensor_copy(gate[:], W_m1i[0:1, 0:1])
            adj_bcg = {}
            for grp in tt_groups:
                r0, gl = grp[0], len(grp)
                is_s = rpath[r0] == 'S'
                dt_, src_ = (bf16, adj_d) if is_s else (fp8, adj8_d)
                tl = singles.tile([128, gl * N], dt_, name=f"adjg{r0}", tag=f"adjg{r0}")
                bc_in = bass.AP(
                    tensor=src_, offset=r0 * N,
                    ap=[[0, 128], [N, gl], [1, N]],
                )
                nc.gpsimd.dma_start(out=tl[:], in_=bc_in)
                adj_bcg[r0] = tl

            # prime the sigmoid/tanh activation table while ScalarE is idle
            actwu = singles.tile([1, 2], f32, name="actwu", tag="actwu")
            nc.vector.memset(actwu[:], 0.0)
            nc.scalar.activation(actwu[:, 0:1], actwu[:, 1:2], mybir.ActivationFunctionType.Sigmoid)
            nc.scalar.activation(actwu[:, 0:1], actwu[:, 1:2], mybir.ActivationFunctionType.Tanh)

            # ---------------- preprocess: h0 (f32 for accuracy) -------------
            p1 = [work.tile([128, N], f32, name=f"p1_{ht}", tag=f"p1_{ht}") for ht in range(HT)]
            for ht in range(HT):
                ps = psp.tile([128, 512], f32, name="ps", tag="ps")
                nc.tensor.matmul(ps[:, 0:N], Wpre1[:, ht * 128:(ht + 1) * 128], xT_sb[:], start=True, stop=True)
                nc.scalar.activation(p1[ht][:], ps[:, 0:N], mybir.ActivationFunctionType.Relu, bias=preb1[:, ht:ht + 1])
            hTf = [singles.tile([128, N], f32, name=f"hTf{ht}", tag=f"hTf{ht}") for ht in range(HT)]
            hTb = [singles.tile([128, N], bf16, name=f"hTbb{ht}", tag=f"hTbb{ht}") for ht in range(HT)]
            for ht in range(HT):
                ps = psp.tile([128, 512], f32, name="ps", tag="ps")
                for kt in range(HT):
                    nc.tensor.matmul(ps[:, 0:N], W_pre2[:, kt * H + ht * 128: kt * H + (ht + 1) * 128], p1[kt][:], start=(kt == 0), stop=(kt == HT - 1))
                nc.scalar.activation(hTf[ht][:], ps[:, 0:N], mybir.ActivationFunctionType.Identity, bias=preb2[:, ht:ht + 1])
                nc.vector.tensor_copy(hTb[ht][:], hTf[ht][:])

            # ---------------- message passing iterations ----------------
            for t in range(T):
                # hiT = (h_loc @ W1_i).T [h, i] f32 (bias-free; bf16 matmul)
                hiTf = [work.tile([128, NLOC], f32, name=f"hiTf{ht}", tag=f"hiTf{ht}") for ht in range(HT)]
                for ht in range(HT):
                    ps = psp.tile([128, 512], f32, name="ps", tag="ps")
                    for kt in range(HT):
                        nc.tensor.matmul(ps[:, 0:NLOC], W_m1i[:, kt * H + ht * 128: kt * H + (ht + 1) * 128], hTb[kt][:, 0:NLOC], start=(kt == 0), stop=(kt == HT - 1))
                    nc.scalar.activation(hiTf[ht][:], ps[:, 0:NLOC], mybir.ActivationFunctionType.Identity)
                # hjbT = (h @ W1_j + b1).T [h, j] bf16; local half only here.
                # Partner half is emitted after the leads-local fused ops so
                # the DVE queue is not blocked behind the exchange subtracts.
                hjbT = [work.tile([128, N], bf16, name=f"hjbT{ht}", tag=f"hjbT{ht}") for ht in range(HT)]
                psj = []
                for ht in range(HT):
                    ps = psp.tile([128, 512], f32, name="ps", tag="ps")
                    psj.append(ps)
                    for kt in range(HT):
                        nc.tensor.matmul(ps[:, 0:NLOC], W_m1j[:, kt * H + ht * 128: kt * H + (ht + 1) * 128], hTb[kt][:, 0:NLOC], start=(kt == 0), stop=(kt == HT - 1))
                    nc.scalar.activation(hjbT[ht][:, 0:NLOC], ps[:, 0:NLOC], mybir.ActivationFunctionType.Identity, bias=msgb1[:, ht:ht + 1])

                ps_rz = psh.tile([128, 512], f32, name="ps_rz", tag="ps_rz")
                ps_gh = psh.tile([128, 512], f32, name="ps_gh", tag="ps_gh")

                aggT = [work.tile([128, NLOC], f32, name=f"aggT{ht}", tag=f"aggT{ht}") for ht in range(HT)]
                rz_sb = work.tile([128, 512], f32, name="rz_sb", tag="rz_sb")

                def emit_fused(i, split=None):
                    """Fused e-ops for receiver i (both h-tiles). split:
                    None = whole row; 'local'/'partner' = j-half with accum
                    chaining via the s1 seed."""
                    for ht in range(HT):
                        scr = eloop.tile([128, N], fp8, name="scr", tag="scr")
                        a01 = adj_bcg[(i // SB) * SB]
                        aoff = (i % SB) * N
                        if split == 'local':
                            nc.vector._custom_dve(
                                FUSED, out=scr[:, 0:NLOC],
                                in0=hjbT[ht][:, 0:NLOC],
                                in1=a01[:, aoff:aoff + NLOC],
                                s0=hiTf[ht][:, i:i + 1], s1=0.0,
                                accum_out=aggT[ht][:, i:i + 1])
                        elif split == 'partner':
                            nc.vector._custom_dve(
                                FUSED, out=scr[:, NLOC:N],
                                in0=hjbT[ht][:, NLOC:N],
                                in1=a01[:, aoff + NLOC:aoff + N],
                                s0=hiTf[ht][:, i:i + 1],
                                s1=aggT[ht][:, i:i + 1],
                                accum_out=aggT[ht][:, i:i + 1])
                        else:
                            nc.vector._custom_dve(
                                FUSED, out=scr[:],
                                in0=hjbT[ht][:],
                                in1=a01[:, aoff:aoff + N],
                                s0=hiTf[ht][:, i:i + 1], s1=0.0,
                                accum_out=aggT[ht][:, i:i + 1])

                def emit_sc_group(grp):
                    """GPS mask + ScalarE relu-bias-accum for 8 receivers."""
                    r0 = grp[0]
                    for ht in range(HT):
                        hjb_rep = bass.AP(
                            tensor=hjbT[ht].tensor, offset=hjbT[ht].offset,
                            ap=[hjbT[ht].ap[0], [0, len(grp)], [1, N]])
                        w = eloop.tile([128, SB * N], bf16, name="w", tag="w")
                        nc.vector.tensor_tensor(
                            out=w[:, 0:len(grp) * N], in0=hjb_rep,
                            in1=adj_bcg[r0][:], op=mybir.AluOpType.add)
                        for k, i in enumerate(grp):
                            scr = eloop.tile([128, N], bf16, name="scs", tag="scs")
                            nc.scalar.activation(
                                scr[:], w[:, k * N:(k + 1) * N],
                                mybir.ActivationFunctionType.Relu,
                                bias=hiTf[ht][:, i:i + 1],
                                accum_out=aggT[ht][:, i:i + 1])

                def emit_gru(c):
                    C0_, C1_ = CHB[c]
                    CWc = C1_ - C0_
                    # msgT = W2m.T @ agg + deg * b2  (bf16 matmuls)
                    ps_m = psp.tile([128, 512], f32, name="ps", tag="ps")
                    for ht in range(HT):
                        for kt in range(HT):
                            nc.tensor.matmul(ps_m[:, ht * CWc:(ht + 1) * CWc], W_m2[:, kt * H + ht * 128: kt * H + (ht + 1) * 128], aggT[kt][:, C0_:C1_], start=(kt == 0), stop=False)
                        nc.tensor.matmul(ps_m[:, ht * CWc:(ht + 1) * CWc], msgb2[0:1, ht * 128:(ht + 1) * 128], deg_row[0:1, C0_:C1_], start=False, stop=True)
                    msgTb = work.tile([128, 2 * 64], bf16, name="msgTb", tag="msgTb")
                    for ht in range(HT):
                        nc.vector.tensor_copy(msgTb[:, ht * CWc:(ht + 1) * CWc], ps_m[:, ht * CWc:(ht + 1) * CWc])
                    # gate matmuls (bf16): Wih then Whh per psum region
                    for mt in range(4):
                        for kt in range(HT):
                            nc.tensor.matmul(ps_rz[:, mt * 128 + C0_: mt * 128 + C1_], W_ih[:, kt * 768 + mt * 128: kt * 768 + (mt + 1) * 128], msgTb[:, kt * CWc:(kt + 1) * CWc], start=(kt == 0), stop=False)
                        for kt in range(HT):
                            nc.tensor.matmul(ps_rz[:, mt * 128 + C0_: mt * 128 + C1_], W_hh[:, kt * 768 + mt * 128: kt * 768 + (mt + 1) * 128], hTb[kt][:, C0_:C1_], start=False, stop=(kt == HT - 1))
                    for ht in range(HT):
                        for kt in range(HT):
                            nc.tensor.matmul(ps_gh[:, ht * 128 + C0_: ht * 128 + C1_], W_ih[:, kt * 768 + (4 + ht) * 128: kt * 768 + (5 + ht) * 128], msgTb[:, kt * CWc:(kt + 1) * CWc], start=(kt == 0), stop=(kt == HT - 1))
                        for kt in range(HT):
                            nc.tensor.matmul(ps_gh[:, 256 + ht * 128 + C0_: 256 + ht * 128 + C1_], W_hh[:, kt * 768 + (4 + ht) * 128: kt * 768 + (5 + ht) * 128], hTb[kt][:, C0_:C1_], start=(kt == 0), stop=(kt == HT - 1))
                    for mt in range(4):
                        nc.scalar.activation(rz_sb[:, mt * 128 + C0_: mt * 128 + C1_], ps_rz[:, mt * 128 + C0_: mt * 128 + C1_], mybir.ActivationFunctionType.Sigmoid, bias=brz[:, mt:mt + 1])
                    for ht in range(HT):
                        # rhn = (gh_n + bhhn) * r
                        rhn = work.tile([128, 80], f32, name="rhn", tag="rhn")[:, 0:CWc]
                        nc.vector.scalar_tensor_tensor(
                            out=rhn, in0=ps_gh[:, 256 + ht * 128 + C0_: 256 + ht * 128 + C1_],
                            scalar=bhhn[:, ht:ht + 1], in1=rz_sb[:, ht * 128 + C0_: ht * 128 + C1_],
                            op0=mybir.AluOpType.add, op1=mybir.AluOpType.mult)
                        nsum = work.tile([128, 80], f32, name="nsum", tag="nsum")[:, 0:CWc]
                        nc.vector.scalar_tensor_tensor(
                            out=nsum, in0=ps_gh[:, ht * 128 + C0_: ht * 128 + C1_],
                            scalar=bihn[:, ht:ht + 1], in1=rhn,
                            op0=mybir.AluOpType.add, op1=mybir.AluOpType.add)
                        n_t = work.tile([128, 80], f32, name="n_t", tag="n_t")[:, 0:CWc]
                        nc.scalar.activation(n_t, nsum, mybir.ActivationFunctionType.Tanh)
                        hmn = work.tile([128, 80], f32, name="hmn", tag="hmn")[:, 0:CWc]
                        nc.gpsimd.tensor_sub(hmn, hTf[ht][:, C0_:C1_], n_t)
                        zh = work.tile([128, 80], f32, name="zh", tag="zh")[:, 0:CWc]
                        nc.gpsimd.tensor_mul(zh, rz_sb[:, 256 + ht * 128 + C0_: 256 + ht * 128 + C1_], hmn)
                        nc.vector.tensor_add(hTf[ht][:, C0_:C1_], n_t, zh)
                        nc.scalar.activation(hTb[ht][:, C0_:C1_], hTf[ht][:, C0_:C1_], mybir.ActivationFunctionType.Identity)
                        if t < T - 1:
                            nc.sync.dma_start(out=cc_in[t][ht * 128:(ht + 1) * 128, C0_:C1_], in_=hTb[ht][:, C0_:C1_])

                # ---- emission order ----
                # 1. leads-local fused (DVE busy while the exchange lands)
                lead_set = set(lead) if t > 0 else set()
                for i in sorted(lead_set):
                    emit_fused(i, split='local')
                # 2. exchange completion: partner h = sum - local (bf16)
                if t > 0:
                    for ht in range(HT):
                        nc.vector.tensor_sub(hTb[ht][:, NLOC:N], rs_prev[ht][:], hTb[ht][:, 0:NLOC])
                # 3. hjbT partner half
                for ht in range(HT):
                    ps = psj[ht]
                    for kt in range(HT):
                        nc.tensor.matmul(ps[:, NLOC:N], W_m1j[:, kt * H + ht * 128: kt * H + (ht + 1) * 128], hTb[kt][:, NLOC:N], start=(kt == 0), stop=(kt == HT - 1))
                    nc.scalar.activation(hjbT[ht][:, NLOC:N], ps[:, NLOC:N], mybir.ActivationFunctionType.Identity, bias=msgb1[:, ht:ht + 1])
                # 4. leads-partner (seeded accum)
                for i in sorted(lead_set):
                    emit_fused(i, split='partner')
                # 5. all S-group masks early (GPS queue) + their SC accums
                for grp in tt_groups:
                    if rpath[grp[0]] == 'S':
                        emit_sc_group(grp)
                # 6. fused receivers chunk0, GRU(0), chunk1, GRU(1)
                for grp in tt_groups:
                    r0 = grp[0]
                    if r0 >= CHB[0][1] or rpath[r0] == 'S':
                        continue
                    for i in grp:
                        if i not in lead_set:
                            emit_fused(i)
                emit_gru(0)
                for grp in tt_groups:
                    r0 = grp[0]
                    if r0 < CHB[0][1] or rpath[r0] == 'S':
                        continue
                    for i in grp:
                        if i not in lead_set:
                            emit_fused(i)
                emit_gru(1)

                if t < T - 1:
                    nc.gpsimd.collective_compute(
                        "AllReduce", mybir.AluOpType.add, replica_groups=groups,
                        ins=[cc_in[t][:]], outs=[cc_out[t][:]])
                    rs_prev = []
                    for ht in range(HT):
                        rs = work.tile([128, NLOC], bf16, name="rs", tag="rs")
                        nc.sync.dma_start(out=rs[:], in_=cc_out[t][ht * 128:(ht + 1) * 128, :])
                        rs_prev.append(rs)

            # ---------------- readout ----------------
            # g as a [1, 256] row so the collective bounce DMAs are single
            # descriptors (partition-major [128,1] DMAs cost ~10us each).
            gT = [work.tile([128, 1], f32, name=f"gT{ht}", tag=f"gT{ht}") for ht in range(HT)]
            grow = work.tile([1, 2 * 128], f32, name="grow", tag="grow")
            ps_g = psp.tile([128, 512], f32, name="ps", tag="ps")
            for ht in range(HT):
                nc.vector.reduce_sum(gT[ht][:], hTf[ht][:, 0:NLOC], axis=mybir.AxisListType.X)
                # transpose [128,1] -> [1,128] via identity-rhs matmul
                nc.tensor.matmul(ps_g[0:1, ht * 128:(ht + 1) * 128], gT[ht][:], identf[:], start=True, stop=True)
            nc.vector.tensor_copy(grow[:], ps_g[0:1, 0:256])
            nc.sync.dma_start(out=gcc_in[:], in_=grow[:])
            nc.gpsimd.collective_compute(
                "AllReduce", mybir.AluOpType.add, replica_groups=groups,
                ins=[gcc_in[:]], outs=[gcc_out[:]])
            gsrow = work.tile([1, 2 * 128], f32, name="gsrow", tag="gsrow")
            nc.sync.dma_start(out=gsrow[:], in_=gcc_out[:])
            gs = [work.tile([128, 1], f32, name=f"gs{ht}", tag=f"gs{ht}") for ht in range(HT)]
            ps_g2 = psp.tile([128, 512], f32, name="ps", tag="ps")
            for ht in range(HT):
                # [1,128] row -> [128,1] column via K=1 outer with ones
                nc.tensor.matmul(ps_g2[0:128, ht:ht + 1], gsrow[0:1, ht * 128:(ht + 1) * 128], ones11[0:1, 0:1], start=True, stop=True)
                nc.vector.tensor_copy(gs[ht][:], ps_g2[:, ht:ht + 1])
            y1 = [work.tile([128, 1], f32, name=f"y1{ht}", tag=f"y1{ht}") for ht in range(HT)]
            for ht in range(HT):
                ps = psp.tile([128, 512], f32, name="ps", tag="ps")
                for kt in range(HT):
                    nc.tensor.matmul(ps[:, 0:1], W_ro1[:, kt * H + ht * 128: kt * H + (ht + 1) * 128], gs[kt][:], start=(kt == 0), stop=(kt == HT - 1))
                nc.scalar.activation(y1[ht][:], ps[:, 0:1], mybir.ActivationFunctionType.Relu, bias=rob1[:, ht:ht + 1])
            ps_q = psp.tile([128, 512], f32, name="ps", tag="ps")
            for kt in range(HT):
                nc.tensor.matmul(ps_q[0:A, 0:1], W_ro2[:, kt * A:(kt + 1) * A], y1[kt][:], start=(kt == 0), stop=(kt == HT - 1))
            q_sb = work.tile([A, 1], f32, name="q_sb", tag="q_sb")
            nc.scalar.activation(q_sb[:], ps_q[0:A, 0:1], mybir.ActivationFunctionType.Identity, bias=rob2[:])
            nc.sync.dma_start(out=q_out[:], in_=q_sb[:])

    nc.compile()
    return nc


def _in_maps(inputs):
    nf = np.asarray(inputs["node_features"], np.float32)
    adj = np.asarray(inputs["adjacency"])
    msg_W1 = np.asarray(inputs["msg_W1"], np.float32)
    gbih = np.asarray(inputs["gru_bih"], np.float32)
    gbhh = np.asarray(inputs["gru_bhh"], np.float32)

    def cols(v, nt):  # [nt*128] -> [128, nt] partition-major columns
        return np.ascontiguousarray(np.asarray(v, np.float32).reshape(nt, 128).T)

    def wsb(w, dt=np.float32):  # [K, M] -> [128, (K//128)*M]
        w = np.asarray(w, np.float32)
        K, M = w.shape
        return np.ascontiguousarray(
            np.concatenate([w[k * 128:(k + 1) * 128] for k in range(K // 128)], axis=1)
        ).astype(dt)

    # per-receiver mask form: fused groups get 0/1, SC groups get (a-1)*32
    rform = np.zeros(NLOC, np.int32)  # 0 -> adj01, 1 -> adjm32
    for g, p in enumerate(GPATHS):
        if p == 'S':
            rform[g * SB:(g + 1) * SB] = 1

    shared = {
        "pre_W1": np.asarray(inputs["pre_W1"], np.float32),
        "pre_W2": wsb(inputs["pre_W2"]),
        "W1i": wsb(msg_W1[:H], BF16_NP),
        "W1j": wsb(msg_W1[H:], BF16_NP),
        "W2m": wsb(inputs["msg_W2"]),
        "Wihb": wsb(inputs["gru_Wih"], BF16_NP),
        "Whhb": wsb(inputs["gru_Whh"], BF16_NP),
        "roW1": wsb(inputs["ro_W1"]),
        "roW2": wsb(inputs["ro_W2"]),
        "preb1c": cols(inputs["pre_b1"], HT),
        "preb2c": cols(inputs["pre_b2"], HT),
        "msgb1c": cols(inputs["msg_b1"], HT),
        "msgb2r": np.asarray(inputs["msg_b2"], np.float32)[None, :],
        "brzc": cols((gbih + gbhh)[: 2 * H], 4),
        "bihnc": cols(gbih[2 * H:], HT),
        "bhhnc": cols(gbhh[2 * H:], HT),
        "rob1c": cols(inputs["ro_b1"], HT),
        "rob2c": np.asarray(inputs["ro_b2"], np.float32)[:, None],
        "identf": np.eye(128, dtype=np.float32),
    }
    maps = []
    for c in range(8):
        b, half = c // 2, c % 2
        lo, hi = half * NLOC, (half + 1) * NLOC
        perm = np.r_[lo:hi, 0:lo, hi:N]
        m = dict(shared)
        m["xT"] = np.ascontiguousarray(nf[b].T[:, perm])
        a = adj[b, lo:hi][:, perm].astype(np.float32)
        m["adjb"] = ((a - 1) * 32.0).astype(BF16_NP)
        m["adj8"] = a.astype(FP8_NP)
        m["degr"] = adj[b, lo:hi].sum(axis=1).astype(np.float32)[None, :]
        maps.append(m)
    return maps


def kernel(**inputs) -> np.ndarray:
    if "nc" not in _CACHE:
        _CACHE["nc"] = build_program()
    nc = _CACHE["nc"]
    maps = _in_maps(inputs)
    res = run_bass_kernel_spmd(nc, maps, list(range(8))).results
    q = np.stack([res[2 * b]["q_out"][:, 0] for b in range(B)]).astype(np.float32)
    return q
